# revision 22
# baseline (speedup 1.0000x reference)
"""AdaptiveLabelLoss Trainium2 kernel (8 NeuronCores, class-sharded).

loss = mean_b [ lse_b - 0.9*pred[b,t_b] - 0.1*conf[t_b].pred_b ]
where conf is the row-normalized exp cosine-similarity confusion matrix
(diagonal zeroed) and lse is logsumexp over pred rows. The Dirichlet
sample of the reference is replaced by its analytic mean (= conf row),
which matches the fixed-key sample mean to ~2e-5 relative.

Sharding: core k owns classes [512k, 512k+512). Batch rows are routed to
the core that owns their target class, grouped by (local target // 128)
into 4 groups, padded to a uniform number of 128-row tiles. All
core-dependence lives in the data (the program is SPMD-uniform).
"""

import os
import numpy as np
import ml_dtypes

B, C, D = 16384, 4096, 1024
NCORES = 8
CHUNK = C // NCORES          # 512 classes per core
NG = 4                       # groups of 128 local classes
CONFIDENCE = 0.9
SMOOTHING = 0.1
E_CONST = float(np.exp(np.float32(1.0)))  # e^1 = diagonal of exp(sim)

_cache = {}
LAST_RESULTS = None  # for test harness introspection


def _split_multiwait_drains(nc, max_waits: int = 1):
    """Walrus (CoreV3) rejects instructions carrying many sem waits. The
    Tile kernel-tail drain waits on every engine/queue sem at once; split
    the extras onto preceding single-wait drains on the same engine."""
    import concourse.mybir as mybir
    import bass_rust
    for f in nc.m.functions:
        for bb in f.blocks:
            i = 0
            insts = bb.instructions
            while i < len(insts):
                inst = insts[i]
                si = inst.sync_info
                if si is not None and si.on_wait and len(si.on_wait) > max_waits:
                    waits = list(si.on_wait)
                    keep = waits[:max_waits]
                    extra = waits[max_waits:]
                    pre = []
                    for j, w in enumerate(extra):
                        d = mybir.InstDrain(
                            name=f"{inst.name}-sw{j}", ins=[], outs=[])
                        d.engine = inst.engine
                        d.sync_info = bass_rust.SyncInfo(
                            on_wait=[w], on_update=[])
                        pre.append(d)
                    inst.sync_info = bass_rust.SyncInfo(
                        on_wait=keep, on_update=list(si.on_update or []))
                    for j, d in enumerate(pre):
                        insts.insert(i + j, d)
                    i += len(pre)
                i += 1


def _build(nkt: int, stage: str = "full", split_drains: bool = True):
    """Build + compile the SPMD program. nkt = 128-row tiles per group.
    stage: debug knob - "in", "norm", "g", "q", or "full"."""
    import concourse.bass as bass
    import concourse.bacc as bacc
    import concourse.tile as tile
    import concourse.mybir as mybir
    import contextlib

    f32 = mybir.dt.float32
    bf16 = mybir.dt.bfloat16
    AL = mybir.AluOpType
    AF = mybir.ActivationFunctionType

    TK = NG * nkt            # total row tiles
    S = TK * 128             # padded rows per core
    F = TK + 4   # fin columns: TK lse | 4 dot cols

    nc = bacc.Bacc("TRN2", target_bir_lowering=False, debug=False,
                   num_devices=NCORES)

    predb = nc.dram_tensor("predb", [S, C], bf16, kind="ExternalInput").ap()
    wt_all = nc.dram_tensor("wt_all", [D, C], bf16, kind="ExternalInput").ap()
    wt_loc = nc.dram_tensor("wt_loc", [D, CHUNK], bf16, kind="ExternalInput").ap()
    ltg = nc.dram_tensor("ltg", [128, TK], f32, kind="ExternalInput").ap()
    vmask = nc.dram_tensor("vmask", [128, TK], f32, kind="ExternalInput").ap()
    dmask = nc.dram_tensor("dmask", [128, C], bf16, kind="ExternalInput").ap()
    out = nc.dram_tensor("out", [1, 1], f32, kind="ExternalOutput").ap()

    with tile.TileContext(nc) as tc:
        stack = contextlib.ExitStack()
        with stack:
            persist = stack.enter_context(tc.tile_pool(name="persist", bufs=1))
            pred_pool = stack.enter_context(tc.tile_pool(name="pred", bufs=4))
            escr_pool = stack.enter_context(tc.tile_pool(name="escr", bufs=1))
            oh_pool = stack.enter_context(tc.tile_pool(name="oh", bufs=2))
            dram_pool = stack.enter_context(
                tc.tile_pool(name="dram", bufs=1, space="DRAM"))

            # ---- persistent tiles ----
            conf = persist.tile([128, NG * C], bf16)      # 32KB/part
            dmask_sb = persist.tile([128, C], bf16)       # 8KB
            colidx = persist.tile([128, 128], f32)
            ltg_sb = persist.tile([128, TK], f32)
            vmask_sb = persist.tile([128, TK], f32)
            esums = persist.tile([128, TK], f32)
            small = persist.tile([128, 384], f32)
            nrow = persist.tile([1, C], f32)   # col-form norms^2 -> 1/norm
            # small col map (f32 scratch columns):
            NSQL = 0                   # [0,4)   local norms^2
            INVL = 4                   # [4,8)   local 1/norm (col m)
            RDEN = 8                   # [8,12)  1/(rowsum-e) per group
            SPR = 12                   # [12,16) 9 - e*rden per group
            NSQ32 = 320                # [320,352) global norms^2 (row form)
            ESG = 16                   # [16,48) exp-sim row sums (m*8+n)
            FIN = 48                   # [48,48+F)
            COEF = FIN + F + 4         # [.., +F)
            LNV = COEF + F + 4         # [.., +TK)
            PROD = LNV + TK + 4        # [.., +F)
            ONES = PROD + F + 2
            OUTC = ONES + 2
            assert OUTC < 384

            nc.sync.dma_start(ltg_sb[:], ltg)
            nc.sync.dma_start(vmask_sb[:], vmask)
            nc.sync.dma_start(dmask_sb[:], dmask)
            nc.gpsimd.iota(colidx[:], pattern=[[1, 128]], base=0,
                           channel_multiplier=0,
                           allow_small_or_imprecise_dtypes=True)
            nc.vector.memset(small[:, ONES:ONES + 1], 1.0)

            if stage != "in":
                # ============== Phase A: norms + Phase B: conf ==============
                with tc.tile_pool(name="wtp", bufs=1) as wtp, \
                     tc.tile_pool(name="gscr", bufs=2) as gscr, \
                     tc.tile_pool(name="wsq", bufs=2) as wsqp, \
                     tc.tile_pool(name="psA", bufs=4, space="PSUM") as psA:

                    wt_sb = wtp.tile([128, 8 * C], bf16)     # W^T, kd-sliced
                    wtloc_sb = wtp.tile([128, 8 * CHUNK], bf16)
                    invb = wtp.tile([128, C], bf16)          # 1/norm bcast

                    for kd in range(8):
                        nc.sync.dma_start(wt_sb[:, kd * C:(kd + 1) * C],
                                          wt_all[kd * 128:(kd + 1) * 128, :])
                        nc.sync.dma_start(
                            wtloc_sb[:, kd * CHUNK:(kd + 1) * CHUNK],
                            wt_loc[kd * 128:(kd + 1) * 128, :])

                    # identity for PE transpose / diag extraction
                    ident = gscr.tile([128, 128], f32, tag="ident")
                    pidx = gscr.tile([128, 1], f32, tag="pidx")
                    nc.gpsimd.iota(pidx[:], pattern=[[0, 1]], base=0,
                                   channel_multiplier=1,
                                   allow_small_or_imprecise_dtypes=True)
                    nc.vector.tensor_scalar(ident[:], colidx[:], pidx[:],
                                            None, op0=AL.is_equal)

                    # norms^2 via Gram-diagonal blocks on PE (row form):
                    # global from wt_sb (32 blocks), local from wtloc_sb (4)
                    for j in range(32):
                        gb = psA.tile([128, 128], f32, tag="gps",
                                      name=f"gnb{j}")
                        for kd in range(8):
                            sl = wt_sb[:, kd * C + 128 * j:
                                       kd * C + 128 * j + 128]
                            nc.tensor.matmul(gb[:], sl, sl,
                                             start=(kd == 0), stop=(kd == 7))
                        db = gscr.tile([128, 128], f32, tag="db")
                        nc.vector.tensor_tensor(db[:], gb[:], ident[:],
                                                op=AL.mult)
                        nc.vector.reduce_sum(
                            small[:, NSQ32 + j:NSQ32 + j + 1], db[:],
                            axis=mybir.AxisListType.X)
                    for m in range(NG):
                        gb = psA.tile([128, 128], f32, tag="gps",
                                      name=f"lnb{m}")
                        for kd in range(8):
                            sl = wtloc_sb[:, kd * CHUNK + 128 * m:
                                          kd * CHUNK + 128 * m + 128]
                            nc.tensor.matmul(gb[:], sl, sl,
                                             start=(kd == 0), stop=(kd == 7))
                        db = gscr.tile([128, 128], f32, tag="db")
                        nc.vector.tensor_tensor(db[:], gb[:], ident[:],
                                                op=AL.mult)
                        nc.vector.reduce_sum(
                            small[:, NSQL + m:NSQL + m + 1], db[:],
                            axis=mybir.AxisListType.X)

                    # inv = 1/max(sqrt(nsq), eps), cheap on row form
                    nc.scalar.activation(small[:, NSQ32:NSQ32 + 32],
                                         small[:, NSQ32:NSQ32 + 32], AF.Sqrt)
                    nc.vector.tensor_scalar_max(small[:, NSQ32:NSQ32 + 32],
                                                small[:, NSQ32:NSQ32 + 32],
                                                1e-8)
                    nc.vector.reciprocal(small[:, NSQ32:NSQ32 + 32],
                                         small[:, NSQ32:NSQ32 + 32])
                    nc.scalar.activation(small[:, INVL:INVL + 4],
                                         small[:, NSQL:NSQL + 4], AF.Sqrt)
                    nc.vector.tensor_scalar_max(small[:, INVL:INVL + 4],
                                                small[:, INVL:INVL + 4], 1e-8)
                    nc.vector.reciprocal(small[:, INVL:INVL + 4],
                                         small[:, INVL:INVL + 4])

                    # inv32 -> [32,128] -> DRAM [4096] -> bcast [128, C]
                    tp = psA.tile([32, 128], f32, tag="gps", name="tpp")
                    nc.tensor.transpose(tp[:], small[:, NSQ32:NSQ32 + 32],
                                        ident[:])
                    tps = gscr.tile([32, 128], f32, tag="tps")
                    nc.scalar.copy(tps[:], tp[:])
                    nrmd = dram_pool.tile([C], f32)
                    nc.sync.dma_start(
                        nrmd[:].rearrange("(j p) -> j p", p=128), tps[:])
                    nr = nrmd[:]
                    nc.gpsimd.dma_start(invb[:], bass.AP(
                        tensor=nr.tensor, offset=nr.offset,
                        ap=[[0, 128]] + [list(p) for p in nr.ap]))

                    # ---- Phase B: sim chunk -> conf (PE -> ACT direct) ----
                    if stage != "norm":
                        for m in range(NG):
                            tmpm = gscr.tile([128, C], bf16, tag="gtmp")
                            for n in range(8):
                                g_ps = psA.tile([128, 512], f32, tag="gps")
                                for kd in range(8):
                                    nc.tensor.matmul(
                                        g_ps[:],
                                        wtloc_sb[:, kd * CHUNK + 128 * m:
                                                 kd * CHUNK + 128 * m + 128],
                                        wt_sb[:, kd * C + 512 * n:
                                              kd * C + 512 * n + 512],
                                        start=(kd == 0), stop=(kd == 7))
                                nc.vector.tensor_tensor(
                                    tmpm[:, 512 * n:512 * n + 512], g_ps[:],
                                    invb[:, 512 * n:512 * n + 512],
                                    op=AL.mult)
                            nc.scalar.activation(
                                conf[:, m * C:(m + 1) * C],
                                tmpm[:], AF.Exp,
                                scale=small[:, INVL + m:INVL + m + 1],
                                accum_out=small[:, ESG + m:ESG + m + 1])
                            # den = rowsum - e ; rden = 1/den
                            nc.vector.tensor_scalar_add(
                                small[:, RDEN + m:RDEN + m + 1],
                                small[:, ESG + m:ESG + m + 1], -E_CONST)
                            nc.vector.reciprocal(
                                small[:, RDEN + m:RDEN + m + 1],
                                small[:, RDEN + m:RDEN + m + 1])
                            # conf = e * rden (diag kept; corrected next)
                            nc.vector.tensor_scalar_mul(
                                conf[:, m * C:(m + 1) * C],
                                conf[:, m * C:(m + 1) * C],
                                small[:, RDEN + m:RDEN + m + 1])
                            # fold -0.9*pred_t and the conf-diagonal
                            # correction in: conf_m += (9 - e*rden)*dmask_sh,
                            # then the dot (coef -0.1) covers all Q terms.
                            nc.vector.tensor_scalar(
                                small[:, SPR + m:SPR + m + 1],
                                small[:, RDEN + m:RDEN + m + 1],
                                -E_CONST, 9.0, op0=AL.mult, op1=AL.add)
                            sdm = escr_pool.tile([128, C], bf16, tag="escr")
                            nc.vector.tensor_scalar(
                                sdm[:, 0:C - 128 * m],
                                dmask_sb[:, 0:C - 128 * m],
                                small[:, SPR + m:SPR + m + 1],
                                None, op0=AL.mult)
                            nc.vector.tensor_tensor(
                                conf[:, m * C + 128 * m:(m + 1) * C],
                                conf[:, m * C + 128 * m:(m + 1) * C],
                                sdm[:, 0:C - 128 * m], op=AL.add)

                # ============== Phase C: Q per group + lse ==============
                if stage not in ("norm", "g"):
                    with tc.tile_pool(name="psQ", bufs=1,
                                      space="PSUM") as psQ, \
                         tc.tile_pool(name="dscr", bufs=2) as dscr_pool:
                        for m in range(NG):
                            q_ps = psQ.tile([128, C], f32)
                            for ki in range(nkt):
                                kt = m * nkt + ki
                                pt = pred_pool.tile([128, C], bf16)
                                nc.sync.dma_start(
                                    pt[:], predb[kt * 128:(kt + 1) * 128, :])
                                oh = oh_pool.tile([128, 128], bf16)
                                nc.vector.tensor_scalar(
                                    oh[:], colidx[:], ltg_sb[:, kt:kt + 1],
                                    None, op0=AL.is_equal)
                                for n in range(8):
                                    nc.tensor.matmul(
                                        q_ps[:, 512 * n:512 * n + 512], oh[:],
                                        pt[:, 512 * n:512 * n + 512],
                                        start=(ki == 0), stop=(ki == nkt - 1))
                                # logsumexp pass: exp in place on the pred
                                # tile (it has no other readers afterwards)
                                nc.scalar.activation(
                                    pt[:], pt[:], AF.Exp,
                                    accum_out=esums[:, kt:kt + 1])
                            # dot with folded conf (covers all Q terms)
                            ds = dscr_pool.tile([128, C], bf16, tag="ds")
                            nc.vector.tensor_tensor(
                                ds[:], q_ps[:], conf[:, m * C:(m + 1) * C],
                                op=AL.mult)
                            nc.vector.reduce_sum(
                                small[:, FIN + TK + m:FIN + TK + m + 1],
                                ds[:], axis=mybir.AxisListType.X)

            # ================= Phase D: final reduction =================
            with tc.tile_pool(name="psF", bufs=1, space="PSUM") as psF:
                outsb = escr_pool.tile([1, 1], f32, tag="outsb")
                if stage == "full":
                    nc.scalar.activation(small[:, LNV:LNV + TK], esums[:],
                                         AF.Ln)
                    nc.vector.tensor_tensor(small[:, FIN:FIN + TK],
                                            small[:, LNV:LNV + TK],
                                            vmask_sb[:], op=AL.mult)
                    nc.vector.memset(small[:, COEF:COEF + TK], 1.0)
                    nc.vector.memset(small[:, COEF + TK:COEF + TK + 4],
                                     -SMOOTHING)
                    nc.vector.tensor_tensor(
                        small[:, PROD:PROD + F],
                        small[:, FIN:FIN + F],
                        small[:, COEF:COEF + F], op=AL.mult)
                    nc.vector.reduce_sum(small[:, OUTC:OUTC + 1],
                                         small[:, PROD:PROD + F],
                                         axis=mybir.AxisListType.X)
                    fps = psF.tile([1, 1], f32)
                    nc.tensor.matmul(fps[:], small[:, OUTC:OUTC + 1],
                                     small[:, ONES:ONES + 1])
                    nc.scalar.copy(outsb[:], fps[:])
                else:
                    nc.vector.memset(outsb[:], 0.0)
                nc.sync.dma_start(out, outsb[:])

    nc.compile()
    if split_drains:
        _split_multiwait_drains(nc)
    return nc


def _prep(pred, weight, target):
    """Host-side sharding/staging. Returns (in_maps, nkt)."""
    pred = np.asarray(pred)
    weight = np.asarray(weight, dtype=np.float32)
    target = np.asarray(target).astype(np.int64)

    w_bf = weight.astype(ml_dtypes.bfloat16)
    wt_bf = np.ascontiguousarray(w_bf.T)

    core_of = (target // CHUNK).astype(np.int64)
    rows_per_core = [np.nonzero(core_of == k)[0] for k in range(NCORES)]

    # group sizes -> uniform tiles per group
    maxg = 1
    groups = []
    for k in range(NCORES):
        lt = target[rows_per_core[k]] - CHUNK * k
        gs = [rows_per_core[k][lt // 128 == m] for m in range(NG)]
        groups.append(gs)
        for g in gs:
            maxg = max(maxg, len(g))
    nkt = (maxg + 127) // 128
    TK = NG * nkt
    S = TK * 128

    pred_bf = pred.astype(ml_dtypes.bfloat16)

    in_maps = []
    for k in range(NCORES):
        predb = np.zeros((S, C), dtype=ml_dtypes.bfloat16)
        ltg = np.full((128, TK), 9999.0, dtype=np.float32)
        vm = np.zeros((128, TK), dtype=np.float32)
        for m in range(NG):
            idx = groups[k][m]
            off = m * nkt * 128
            n = len(idx)
            predb[off:off + n] = pred_bf[idx]
            r = off + np.arange(n)
            ltg[r & 127, r >> 7] = (target[idx] - CHUNK * k - 128 * m)
            vm[r & 127, r >> 7] = 1.0
        dmask = np.zeros((128, C), dtype=ml_dtypes.bfloat16)
        dmask[np.arange(128), CHUNK * k + np.arange(128)] = 1.0
        in_maps.append({
            "predb": predb,
            "wt_all": wt_bf,
            "wt_loc": np.ascontiguousarray(wt_bf[:, CHUNK * k:CHUNK * (k + 1)]),
            "ltg": ltg,
            "vmask": vm,
            "dmask": dmask,
        })
    return in_maps, nkt


def _install_trace_shims():
    """Make trace=True work in containers whose antenv lacks axon_hooks."""
    import sys
    import types
    try:
        import antenv.axon_hooks  # noqa: F401
    except ImportError:
        import antenv
        from trn_agent_boot.trn_boot import _ntff_profile_via_ctypes
        mod = types.ModuleType("antenv.axon_hooks")
        hook = _ntff_profile_via_ctypes("/opt/axon/libaxon_pjrt.so")
        mod.get_axon_ntff_profile_hook = lambda: hook
        mod.set_axon_ntff_profile_hook = lambda h: None
        sys.modules["antenv.axon_hooks"] = mod
        antenv.axon_hooks = mod
    import concourse.bass_utils as bu
    bu.upload_artifacts = lambda tmpdir: "local://" + tmpdir


def kernel(pred, weight, target):
    from concourse.bass_utils import run_bass_kernel_spmd
    global LAST_RESULTS

    in_maps, nkt = _prep(pred, weight, target)
    if nkt not in _cache:
        _cache[nkt] = _build(nkt)
    nc = _cache[nkt]

    trace = bool(int(os.environ.get("AKL_TRACE", "0")))
    if trace:
        _install_trace_shims()
    res = run_bass_kernel_spmd(nc, in_maps, core_ids=list(range(NCORES)),
                               trace=trace)
    LAST_RESULTS = res
    total = np.float64(0.0)
    for k in range(NCORES):
        total += np.float64(res.results[k]["out"][0, 0])
    return np.float32(total / B)


# revision 23
# speedup vs baseline: 1.1974x; 1.1974x over previous
"""AdaptiveLabelLoss Trainium2 kernel (8 NeuronCores, class-sharded).

loss = mean_b [ lse_b - 0.9*pred[b,t_b] - 0.1*conf[t_b].pred_b ]
where conf is the row-normalized exp cosine-similarity confusion matrix
(diagonal zeroed) and lse is logsumexp over pred rows. The Dirichlet
sample of the reference is replaced by its analytic mean (= conf row),
which matches the fixed-key sample mean to ~2e-5 relative.

Sharding: core k owns classes [512k, 512k+512). Batch rows are routed to
the core that owns their target class, grouped by (local target // 128)
into 4 groups, padded to a uniform number of 128-row tiles. All
core-dependence lives in the data (the program is SPMD-uniform).
"""

import os
import numpy as np
import ml_dtypes

B, C, D = 16384, 4096, 1024
NCORES = 8
CHUNK = C // NCORES          # 512 classes per core
NG = 4                       # groups of 128 local classes
CONFIDENCE = 0.9
SMOOTHING = 0.1
E_CONST = float(np.exp(np.float32(1.0)))  # e^1 = diagonal of exp(sim)

_cache = {}
LAST_RESULTS = None  # for test harness introspection


def _split_multiwait_drains(nc, max_waits: int = 1):
    """Walrus (CoreV3) rejects instructions carrying many sem waits. The
    Tile kernel-tail drain waits on every engine/queue sem at once; split
    the extras onto preceding single-wait drains on the same engine."""
    import concourse.mybir as mybir
    import bass_rust
    for f in nc.m.functions:
        for bb in f.blocks:
            i = 0
            insts = bb.instructions
            while i < len(insts):
                inst = insts[i]
                si = inst.sync_info
                if si is not None and si.on_wait and len(si.on_wait) > max_waits:
                    waits = list(si.on_wait)
                    keep = waits[:max_waits]
                    extra = waits[max_waits:]
                    pre = []
                    for j, w in enumerate(extra):
                        d = mybir.InstDrain(
                            name=f"{inst.name}-sw{j}", ins=[], outs=[])
                        d.engine = inst.engine
                        d.sync_info = bass_rust.SyncInfo(
                            on_wait=[w], on_update=[])
                        pre.append(d)
                    inst.sync_info = bass_rust.SyncInfo(
                        on_wait=keep, on_update=list(si.on_update or []))
                    for j, d in enumerate(pre):
                        insts.insert(i + j, d)
                    i += len(pre)
                i += 1


def _build(nkt: int, stage: str = "full", split_drains: bool = True):
    """Build + compile the SPMD program. nkt = 128-row tiles per group.
    stage: debug knob - "in", "norm", "g", "q", or "full"."""
    import concourse.bass as bass
    import concourse.bacc as bacc
    import concourse.tile as tile
    import concourse.mybir as mybir
    import contextlib

    f32 = mybir.dt.float32
    bf16 = mybir.dt.bfloat16
    AL = mybir.AluOpType
    AF = mybir.ActivationFunctionType

    TK = NG * nkt            # total row tiles
    S = TK * 128             # padded rows per core
    F = TK + 4   # fin columns: TK lse | 4 dot cols

    nc = bacc.Bacc("TRN2", target_bir_lowering=False, debug=False,
                   num_devices=NCORES)

    predb = nc.dram_tensor("predb", [S, C], bf16, kind="ExternalInput").ap()
    wt_all = nc.dram_tensor("wt_all", [D, C], bf16, kind="ExternalInput").ap()
    wt_loc = nc.dram_tensor("wt_loc", [D, CHUNK], bf16, kind="ExternalInput").ap()
    ltg = nc.dram_tensor("ltg", [128, TK], f32, kind="ExternalInput").ap()
    vmask = nc.dram_tensor("vmask", [128, TK], f32, kind="ExternalInput").ap()
    dmask = nc.dram_tensor("dmask", [128, C], bf16, kind="ExternalInput").ap()
    out = nc.dram_tensor("out", [1, 1], f32, kind="ExternalOutput").ap()

    with tile.TileContext(nc) as tc:
        stack = contextlib.ExitStack()
        with stack:
            persist = stack.enter_context(tc.tile_pool(name="persist", bufs=1))
            pred_pool = stack.enter_context(tc.tile_pool(name="pred", bufs=4))
            escr_pool = stack.enter_context(tc.tile_pool(name="escr", bufs=2))
            oh_pool = stack.enter_context(tc.tile_pool(name="oh", bufs=2))
            dram_pool = stack.enter_context(
                tc.tile_pool(name="dram", bufs=1, space="DRAM"))

            # ---- persistent tiles ----
            conf = persist.tile([128, NG * C], bf16)      # 32KB/part
            dmask_sb = persist.tile([128, C], bf16)       # 8KB
            colidx = persist.tile([128, 128], f32)
            ltg_sb = persist.tile([128, TK], f32)
            vmask_sb = persist.tile([128, TK], f32)
            esums = persist.tile([128, TK], f32)
            small = persist.tile([128, 384], f32)
            nrow = persist.tile([1, C], f32)   # col-form norms^2 -> 1/norm
            # small col map (f32 scratch columns):
            NSQL = 0                   # [0,4)   local norms^2
            INVL = 4                   # [4,8)   local 1/norm (col m)
            RDEN = 8                   # [8,12)  1/(rowsum-e) per group
            SPR = 12                   # [12,16) 9 - e*rden per group
            NSQ32 = 320                # [320,352) global norms^2 (row form)
            ESG = 16                   # [16,48) exp-sim row sums (m*8+n)
            FIN = 48                   # [48,48+F)
            COEF = FIN + F + 4         # [.., +F)
            LNV = COEF + F + 4         # [.., +TK)
            PROD = LNV + TK + 4        # [.., +F)
            ONES = PROD + F + 2
            OUTC = ONES + 2
            assert OUTC < 384

            nc.sync.dma_start(ltg_sb[:], ltg)
            nc.sync.dma_start(vmask_sb[:], vmask)
            nc.sync.dma_start(dmask_sb[:], dmask)
            nc.gpsimd.iota(colidx[:], pattern=[[1, 128]], base=0,
                           channel_multiplier=0,
                           allow_small_or_imprecise_dtypes=True)
            nc.vector.memset(small[:, ONES:ONES + 1], 1.0)

            if stage != "in":
                # ============== Phase A: norms + Phase B: conf ==============
                with tc.tile_pool(name="wtp", bufs=1) as wtp, \
                     tc.tile_pool(name="gscr", bufs=2) as gscr, \
                     tc.tile_pool(name="wsq", bufs=2) as wsqp, \
                     tc.tile_pool(name="psA", bufs=4, space="PSUM") as psA:

                    wt_sb = wtp.tile([128, 8 * C], bf16)     # W^T, kd-sliced
                    wtloc_sb = wtp.tile([128, 8 * CHUNK], bf16)
                    invb = wtp.tile([128, C], bf16)          # 1/norm bcast

                    for kd in range(8):
                        nc.sync.dma_start(wt_sb[:, kd * C:(kd + 1) * C],
                                          wt_all[kd * 128:(kd + 1) * 128, :])
                        nc.sync.dma_start(
                            wtloc_sb[:, kd * CHUNK:(kd + 1) * CHUNK],
                            wt_loc[kd * 128:(kd + 1) * 128, :])

                    # identity for PE transpose / diag extraction
                    ident = gscr.tile([128, 128], f32, tag="ident")
                    pidx = gscr.tile([128, 1], f32, tag="pidx")
                    nc.gpsimd.iota(pidx[:], pattern=[[0, 1]], base=0,
                                   channel_multiplier=1,
                                   allow_small_or_imprecise_dtypes=True)
                    nc.vector.tensor_scalar(ident[:], colidx[:], pidx[:],
                                            None, op0=AL.is_equal)

                    # norms^2 via Gram-diagonal blocks on PE (row form):
                    # global from wt_sb (32 blocks), local from wtloc_sb (4)
                    for j in range(32):
                        gb = psA.tile([128, 128], f32, tag="gps",
                                      name=f"gnb{j}")
                        for kd in range(8):
                            sl = wt_sb[:, kd * C + 128 * j:
                                       kd * C + 128 * j + 128]
                            nc.tensor.matmul(gb[:], sl, sl,
                                             start=(kd == 0), stop=(kd == 7))
                        db = gscr.tile([128, 128], f32, tag="db")
                        nc.vector.tensor_tensor(db[:], gb[:], ident[:],
                                                op=AL.mult)
                        nc.vector.reduce_sum(
                            small[:, NSQ32 + j:NSQ32 + j + 1], db[:],
                            axis=mybir.AxisListType.X)
                    for m in range(NG):
                        gb = psA.tile([128, 128], f32, tag="gps",
                                      name=f"lnb{m}")
                        for kd in range(8):
                            sl = wtloc_sb[:, kd * CHUNK + 128 * m:
                                          kd * CHUNK + 128 * m + 128]
                            nc.tensor.matmul(gb[:], sl, sl,
                                             start=(kd == 0), stop=(kd == 7))
                        db = gscr.tile([128, 128], f32, tag="db")
                        nc.vector.tensor_tensor(db[:], gb[:], ident[:],
                                                op=AL.mult)
                        nc.vector.reduce_sum(
                            small[:, NSQL + m:NSQL + m + 1], db[:],
                            axis=mybir.AxisListType.X)

                    # inv = 1/max(sqrt(nsq), eps), cheap on row form
                    nc.scalar.activation(small[:, NSQ32:NSQ32 + 32],
                                         small[:, NSQ32:NSQ32 + 32], AF.Sqrt)
                    nc.vector.tensor_scalar_max(small[:, NSQ32:NSQ32 + 32],
                                                small[:, NSQ32:NSQ32 + 32],
                                                1e-8)
                    nc.vector.reciprocal(small[:, NSQ32:NSQ32 + 32],
                                         small[:, NSQ32:NSQ32 + 32])
                    nc.scalar.activation(small[:, INVL:INVL + 4],
                                         small[:, NSQL:NSQL + 4], AF.Sqrt)
                    nc.vector.tensor_scalar_max(small[:, INVL:INVL + 4],
                                                small[:, INVL:INVL + 4], 1e-8)
                    nc.vector.reciprocal(small[:, INVL:INVL + 4],
                                         small[:, INVL:INVL + 4])

                    # inv32 -> [32,128] -> DRAM [4096] -> bcast [128, C]
                    tp = psA.tile([32, 128], f32, tag="gps", name="tpp")
                    nc.tensor.transpose(tp[:], small[:, NSQ32:NSQ32 + 32],
                                        ident[:])
                    tps = gscr.tile([32, 128], f32, tag="tps")
                    nc.scalar.copy(tps[:], tp[:])
                    nrmd = dram_pool.tile([C], f32)
                    nc.sync.dma_start(
                        nrmd[:].rearrange("(j p) -> j p", p=128), tps[:])
                    nr = nrmd[:]
                    nc.gpsimd.dma_start(invb[:], bass.AP(
                        tensor=nr.tensor, offset=nr.offset,
                        ap=[[0, 128]] + [list(p) for p in nr.ap]))

                    # ---- Phase B: sim chunk -> conf (PE -> ACT direct) ----
                    if stage != "norm":
                        for m in range(NG):
                            for n in range(8):
                                g_ps = psA.tile([128, 512], f32, tag="gps")
                                for kd in range(8):
                                    nc.tensor.matmul(
                                        g_ps[:],
                                        wtloc_sb[:, kd * CHUNK + 128 * m:
                                                 kd * CHUNK + 128 * m + 128],
                                        wt_sb[:, kd * C + 512 * n:
                                              kd * C + 512 * n + 512],
                                        start=(kd == 0), stop=(kd == 7))
                                tmp = gscr.tile([128, 512], f32, tag="gtmp")
                                nc.vector.tensor_tensor(
                                    tmp[:], g_ps[:],
                                    invb[:, 512 * n:512 * n + 512],
                                    op=AL.mult)
                                nc.scalar.activation(
                                    conf[:, m * C + 512 * n:
                                         m * C + 512 * n + 512],
                                    tmp[:], AF.Exp,
                                    scale=small[:, INVL + m:INVL + m + 1],
                                    accum_out=small[:, ESG + m * 8 + n:
                                                    ESG + m * 8 + n + 1])
                            # den = rowsum - e ; rden = 1/den
                            nc.vector.reduce_sum(
                                small[:, RDEN + m:RDEN + m + 1],
                                small[:, ESG + m * 8:ESG + m * 8 + 8],
                                axis=mybir.AxisListType.X)
                            nc.vector.tensor_scalar_add(
                                small[:, RDEN + m:RDEN + m + 1],
                                small[:, RDEN + m:RDEN + m + 1], -E_CONST)
                            nc.vector.reciprocal(
                                small[:, RDEN + m:RDEN + m + 1],
                                small[:, RDEN + m:RDEN + m + 1])
                            # conf = e * rden (diag kept; corrected later)
                            nc.vector.tensor_scalar_mul(
                                conf[:, m * C:(m + 1) * C],
                                conf[:, m * C:(m + 1) * C],
                                small[:, RDEN + m:RDEN + m + 1])

                # ============== Phase C: Q per group + lse ==============
                if stage not in ("norm", "g"):
                    with tc.tile_pool(name="psQ", bufs=1,
                                      space="PSUM") as psQ:
                        for m in range(NG):
                            q_ps = psQ.tile([128, C], f32)
                            for ki in range(nkt):
                                kt = m * nkt + ki
                                pt = pred_pool.tile([128, C], bf16)
                                nc.sync.dma_start(
                                    pt[:], predb[kt * 128:(kt + 1) * 128, :])
                                oh = oh_pool.tile([128, 128], bf16)
                                nc.vector.tensor_scalar(
                                    oh[:], colidx[:], ltg_sb[:, kt:kt + 1],
                                    None, op0=AL.is_equal)
                                for n in range(8):
                                    nc.tensor.matmul(
                                        q_ps[:, 512 * n:512 * n + 512], oh[:],
                                        pt[:, 512 * n:512 * n + 512],
                                        start=(ki == 0), stop=(ki == nkt - 1))
                                es = escr_pool.tile([128, C], bf16,
                                                    tag="escr")
                                nc.scalar.activation(
                                    es[:], pt[:], AF.Exp,
                                    accum_out=esums[:, kt:kt + 1])
                            # fold -0.9*pred_t and the conf-diagonal
                            # correction into conf_m: conf_m += s'*dmask_sh
                            # with s' = 9 - e*rden, then the single dot
                            # (coef -0.1) covers all Q terms.
                            nc.vector.tensor_scalar(
                                small[:, SPR + m:SPR + m + 1],
                                small[:, RDEN + m:RDEN + m + 1],
                                -E_CONST, 9.0, op0=AL.mult, op1=AL.add)
                            sdm = escr_pool.tile([128, C], bf16, tag="escr")
                            nc.vector.tensor_scalar(
                                sdm[:, 0:C - 128 * m],
                                dmask_sb[:, 0:C - 128 * m],
                                small[:, SPR + m:SPR + m + 1],
                                None, op0=AL.mult)
                            nc.vector.tensor_tensor(
                                conf[:, m * C + 128 * m:(m + 1) * C],
                                conf[:, m * C + 128 * m:(m + 1) * C],
                                sdm[:, 0:C - 128 * m], op=AL.add)
                            scr2 = escr_pool.tile([128, C], bf16, tag="escr")
                            nc.vector.tensor_tensor(
                                scr2[:], q_ps[:], conf[:, m * C:(m + 1) * C],
                                op=AL.mult)
                            nc.vector.reduce_sum(
                                small[:, FIN + TK + m:FIN + TK + m + 1],
                                scr2[:], axis=mybir.AxisListType.X)

            # ================= Phase D: final reduction =================
            with tc.tile_pool(name="psF", bufs=1, space="PSUM") as psF:
                outsb = escr_pool.tile([1, 1], f32, tag="outsb")
                if stage == "full":
                    nc.scalar.activation(small[:, LNV:LNV + TK], esums[:],
                                         AF.Ln)
                    nc.vector.tensor_tensor(small[:, FIN:FIN + TK],
                                            small[:, LNV:LNV + TK],
                                            vmask_sb[:], op=AL.mult)
                    nc.vector.memset(small[:, COEF:COEF + TK], 1.0)
                    nc.vector.memset(small[:, COEF + TK:COEF + TK + 4],
                                     -SMOOTHING)
                    nc.vector.tensor_tensor(
                        small[:, PROD:PROD + F],
                        small[:, FIN:FIN + F],
                        small[:, COEF:COEF + F], op=AL.mult)
                    nc.vector.reduce_sum(small[:, OUTC:OUTC + 1],
                                         small[:, PROD:PROD + F],
                                         axis=mybir.AxisListType.X)
                    fps = psF.tile([1, 1], f32)
                    nc.tensor.matmul(fps[:], small[:, OUTC:OUTC + 1],
                                     small[:, ONES:ONES + 1])
                    nc.scalar.copy(outsb[:], fps[:])
                else:
                    nc.vector.memset(outsb[:], 0.0)
                nc.sync.dma_start(out, outsb[:])

    nc.compile()
    if split_drains:
        _split_multiwait_drains(nc)
    return nc


def _prep(pred, weight, target):
    """Host-side sharding/staging. Returns (in_maps, nkt)."""
    pred = np.asarray(pred)
    weight = np.asarray(weight, dtype=np.float32)
    target = np.asarray(target).astype(np.int64)

    w_bf = weight.astype(ml_dtypes.bfloat16)
    wt_bf = np.ascontiguousarray(w_bf.T)

    core_of = (target // CHUNK).astype(np.int64)
    rows_per_core = [np.nonzero(core_of == k)[0] for k in range(NCORES)]

    # group sizes -> uniform tiles per group
    maxg = 1
    groups = []
    for k in range(NCORES):
        lt = target[rows_per_core[k]] - CHUNK * k
        gs = [rows_per_core[k][lt // 128 == m] for m in range(NG)]
        groups.append(gs)
        for g in gs:
            maxg = max(maxg, len(g))
    nkt = (maxg + 127) // 128
    TK = NG * nkt
    S = TK * 128

    pred_bf = pred.astype(ml_dtypes.bfloat16)

    in_maps = []
    for k in range(NCORES):
        predb = np.zeros((S, C), dtype=ml_dtypes.bfloat16)
        ltg = np.full((128, TK), 9999.0, dtype=np.float32)
        vm = np.zeros((128, TK), dtype=np.float32)
        for m in range(NG):
            idx = groups[k][m]
            off = m * nkt * 128
            n = len(idx)
            predb[off:off + n] = pred_bf[idx]
            r = off + np.arange(n)
            ltg[r & 127, r >> 7] = (target[idx] - CHUNK * k - 128 * m)
            vm[r & 127, r >> 7] = 1.0
        dmask = np.zeros((128, C), dtype=ml_dtypes.bfloat16)
        dmask[np.arange(128), CHUNK * k + np.arange(128)] = 1.0
        in_maps.append({
            "predb": predb,
            "wt_all": wt_bf,
            "wt_loc": np.ascontiguousarray(wt_bf[:, CHUNK * k:CHUNK * (k + 1)]),
            "ltg": ltg,
            "vmask": vm,
            "dmask": dmask,
        })
    return in_maps, nkt


def _install_trace_shims():
    """Make trace=True work in containers whose antenv lacks axon_hooks."""
    import sys
    import types
    try:
        import antenv.axon_hooks  # noqa: F401
    except ImportError:
        import antenv
        from trn_agent_boot.trn_boot import _ntff_profile_via_ctypes
        mod = types.ModuleType("antenv.axon_hooks")
        hook = _ntff_profile_via_ctypes("/opt/axon/libaxon_pjrt.so")
        mod.get_axon_ntff_profile_hook = lambda: hook
        mod.set_axon_ntff_profile_hook = lambda h: None
        sys.modules["antenv.axon_hooks"] = mod
        antenv.axon_hooks = mod
    import concourse.bass_utils as bu
    bu.upload_artifacts = lambda tmpdir: "local://" + tmpdir


def kernel(pred, weight, target):
    from concourse.bass_utils import run_bass_kernel_spmd
    global LAST_RESULTS

    in_maps, nkt = _prep(pred, weight, target)
    if nkt not in _cache:
        _cache[nkt] = _build(nkt)
    nc = _cache[nkt]

    trace = bool(int(os.environ.get("AKL_TRACE", "0")))
    if trace:
        _install_trace_shims()
    res = run_bass_kernel_spmd(nc, in_maps, core_ids=list(range(NCORES)),
                               trace=trace)
    LAST_RESULTS = res
    total = np.float64(0.0)
    for k in range(NCORES):
        total += np.float64(res.results[k]["out"][0, 0])
    return np.float32(total / B)


# revision 25
# speedup vs baseline: 1.2703x; 1.0609x over previous
"""AdaptiveLabelLoss Trainium2 kernel (8 NeuronCores, class-sharded).

loss = mean_b [ lse_b - 0.9*pred[b,t_b] - 0.1*conf[t_b].pred_b ]
where conf is the row-normalized exp cosine-similarity confusion matrix
(diagonal zeroed) and lse is logsumexp over pred rows. The Dirichlet
sample of the reference is replaced by its analytic mean (= conf row),
which matches the fixed-key sample mean to ~2e-5 relative.

Sharding: core k owns classes [512k, 512k+512). Batch rows are routed to
the core that owns their target class, grouped by (local target // 128)
into 4 groups, padded to a uniform number of 128-row tiles. All
core-dependence lives in the data (the program is SPMD-uniform).
"""

import os
import numpy as np
import ml_dtypes

B, C, D = 16384, 4096, 1024
NCORES = 8
CHUNK = C // NCORES          # 512 classes per core
NG = 4                       # groups of 128 local classes
CONFIDENCE = 0.9
SMOOTHING = 0.1
E_CONST = float(np.exp(np.float32(1.0)))  # e^1 = diagonal of exp(sim)

_cache = {}
LAST_RESULTS = None  # for test harness introspection


def _split_multiwait_drains(nc, max_waits: int = 1):
    """Walrus (CoreV3) rejects instructions carrying many sem waits. The
    Tile kernel-tail drain waits on every engine/queue sem at once; split
    the extras onto preceding single-wait drains on the same engine."""
    import concourse.mybir as mybir
    import bass_rust
    for f in nc.m.functions:
        for bb in f.blocks:
            i = 0
            insts = bb.instructions
            while i < len(insts):
                inst = insts[i]
                si = inst.sync_info
                if si is not None and si.on_wait and len(si.on_wait) > max_waits:
                    waits = list(si.on_wait)
                    keep = waits[:max_waits]
                    extra = waits[max_waits:]
                    pre = []
                    for j, w in enumerate(extra):
                        d = mybir.InstDrain(
                            name=f"{inst.name}-sw{j}", ins=[], outs=[])
                        d.engine = inst.engine
                        d.sync_info = bass_rust.SyncInfo(
                            on_wait=[w], on_update=[])
                        pre.append(d)
                    inst.sync_info = bass_rust.SyncInfo(
                        on_wait=keep, on_update=list(si.on_update or []))
                    for j, d in enumerate(pre):
                        insts.insert(i + j, d)
                    i += len(pre)
                i += 1


def _build(nkt: int, stage: str = "full", split_drains: bool = True,
           psa_bufs: int = 8, exp_halves: bool = False,
           q_split: bool = False, oh_gpsimd: bool = False):
    """Build + compile the SPMD program. nkt = 128-row tiles per group.
    stage: debug knob - "in", "norm", "g", "q", or "full"."""
    import concourse.bass as bass
    import concourse.bacc as bacc
    import concourse.tile as tile
    import concourse.mybir as mybir
    import contextlib

    f32 = mybir.dt.float32
    bf16 = mybir.dt.bfloat16
    AL = mybir.AluOpType
    AF = mybir.ActivationFunctionType

    TK = NG * nkt            # total row tiles
    S = TK * 128             # padded rows per core
    F = TK + 4   # fin columns: TK lse | 4 dot cols

    nc = bacc.Bacc("TRN2", target_bir_lowering=False, debug=False,
                   num_devices=NCORES)

    predb = nc.dram_tensor("predb", [S, C], bf16, kind="ExternalInput").ap()
    wt_all = nc.dram_tensor("wt_all", [D, C], bf16, kind="ExternalInput").ap()
    wt_loc = nc.dram_tensor("wt_loc", [D, CHUNK], bf16, kind="ExternalInput").ap()
    ltg = nc.dram_tensor("ltg", [128, TK], f32, kind="ExternalInput").ap()
    vmask = nc.dram_tensor("vmask", [128, TK], f32, kind="ExternalInput").ap()
    dmask = nc.dram_tensor("dmask", [128, C], bf16, kind="ExternalInput").ap()
    out = nc.dram_tensor("out", [1, 1], f32, kind="ExternalOutput").ap()

    with tile.TileContext(nc) as tc:
        stack = contextlib.ExitStack()
        with stack:
            persist = stack.enter_context(tc.tile_pool(name="persist", bufs=1))
            pred_pool = stack.enter_context(tc.tile_pool(name="pred", bufs=4))
            escr_pool = stack.enter_context(tc.tile_pool(name="escr", bufs=2))
            oh_pool = stack.enter_context(tc.tile_pool(name="oh", bufs=2))
            dram_pool = stack.enter_context(
                tc.tile_pool(name="dram", bufs=1, space="DRAM"))

            # ---- persistent tiles ----
            conf = persist.tile([128, NG * C], bf16)      # 32KB/part
            dmask_sb = persist.tile([128, C], bf16)       # 8KB
            colidx = persist.tile([128, 128], f32)
            ltg_sb = persist.tile([128, TK], f32)
            vmask_sb = persist.tile([128, TK], f32)
            esums = persist.tile([128, TK], f32)
            small = persist.tile([128, 384], f32)
            nrow = persist.tile([1, C], f32)   # col-form norms^2 -> 1/norm
            # small col map (f32 scratch columns):
            NSQL = 0                   # [0,4)   local norms^2
            INVL = 4                   # [4,8)   local 1/norm (col m)
            RDEN = 8                   # [8,12)  1/(rowsum-e) per group
            SPR = 12                   # [12,16) 9 - e*rden per group
            NSQ32 = 320                # [320,352) global norms^2 (row form)
            ESG = 16                   # [16,48) exp-sim row sums (m*8+n)
            FIN = 48                   # [48,48+F)
            COEF = FIN + F + 4         # [.., +F)
            LNV = COEF + F + 4         # [.., +TK)
            PROD = LNV + TK + 4        # [.., +F)
            ONES = PROD + F + 2
            OUTC = ONES + 2
            assert OUTC < 384

            nc.sync.dma_start(ltg_sb[:], ltg)
            nc.sync.dma_start(vmask_sb[:], vmask)
            nc.sync.dma_start(dmask_sb[:], dmask)
            nc.gpsimd.iota(colidx[:], pattern=[[1, 128]], base=0,
                           channel_multiplier=0,
                           allow_small_or_imprecise_dtypes=True)
            nc.vector.memset(small[:, ONES:ONES + 1], 1.0)

            if stage != "in":
                # ============== Phase A: norms + Phase B: conf ==============
                with tc.tile_pool(name="wtp", bufs=1) as wtp, \
                     tc.tile_pool(name="gscr", bufs=2) as gscr, \
                     tc.tile_pool(name="wsq", bufs=2) as wsqp, \
                     tc.tile_pool(name="psA", bufs=psa_bufs, space="PSUM") as psA:

                    wt_sb = wtp.tile([128, 8 * C], bf16)     # W^T, kd-sliced
                    wtloc_sb = wtp.tile([128, 8 * CHUNK], bf16)
                    invb = wtp.tile([128, C], bf16)          # 1/norm bcast

                    for kd in range(8):
                        nc.sync.dma_start(wt_sb[:, kd * C:(kd + 1) * C],
                                          wt_all[kd * 128:(kd + 1) * 128, :])
                        nc.sync.dma_start(
                            wtloc_sb[:, kd * CHUNK:(kd + 1) * CHUNK],
                            wt_loc[kd * 128:(kd + 1) * 128, :])

                    # identity for PE transpose / diag extraction
                    ident = gscr.tile([128, 128], f32, tag="ident")
                    pidx = gscr.tile([128, 1], f32, tag="pidx")
                    nc.gpsimd.iota(pidx[:], pattern=[[0, 1]], base=0,
                                   channel_multiplier=1,
                                   allow_small_or_imprecise_dtypes=True)
                    nc.vector.tensor_scalar(ident[:], colidx[:], pidx[:],
                                            None, op0=AL.is_equal)

                    # norms^2 via Gram-diagonal blocks on PE (row form):
                    # global from wt_sb (32 blocks), local from wtloc_sb (4)
                    for j in range(32):
                        gb = psA.tile([128, 128], f32, tag="gps",
                                      name=f"gnb{j}")
                        for kd in range(8):
                            sl = wt_sb[:, kd * C + 128 * j:
                                       kd * C + 128 * j + 128]
                            nc.tensor.matmul(gb[:], sl, sl,
                                             start=(kd == 0), stop=(kd == 7))
                        db = gscr.tile([128, 128], f32, tag="db")
                        nc.vector.tensor_tensor(db[:], gb[:], ident[:],
                                                op=AL.mult)
                        nc.vector.reduce_sum(
                            small[:, NSQ32 + j:NSQ32 + j + 1], db[:],
                            axis=mybir.AxisListType.X)
                    for m in range(NG):
                        gb = psA.tile([128, 128], f32, tag="gps",
                                      name=f"lnb{m}")
                        for kd in range(8):
                            sl = wtloc_sb[:, kd * CHUNK + 128 * m:
                                          kd * CHUNK + 128 * m + 128]
                            nc.tensor.matmul(gb[:], sl, sl,
                                             start=(kd == 0), stop=(kd == 7))
                        db = gscr.tile([128, 128], f32, tag="db")
                        nc.vector.tensor_tensor(db[:], gb[:], ident[:],
                                                op=AL.mult)
                        nc.vector.reduce_sum(
                            small[:, NSQL + m:NSQL + m + 1], db[:],
                            axis=mybir.AxisListType.X)

                    # inv = 1/max(sqrt(nsq), eps), cheap on row form
                    nc.scalar.activation(small[:, NSQ32:NSQ32 + 32],
                                         small[:, NSQ32:NSQ32 + 32], AF.Sqrt)
                    nc.vector.tensor_scalar_max(small[:, NSQ32:NSQ32 + 32],
                                                small[:, NSQ32:NSQ32 + 32],
                                                1e-8)
                    nc.vector.reciprocal(small[:, NSQ32:NSQ32 + 32],
                                         small[:, NSQ32:NSQ32 + 32])
                    nc.scalar.activation(small[:, INVL:INVL + 4],
                                         small[:, NSQL:NSQL + 4], AF.Sqrt)
                    nc.vector.tensor_scalar_max(small[:, INVL:INVL + 4],
                                                small[:, INVL:INVL + 4], 1e-8)
                    nc.vector.reciprocal(small[:, INVL:INVL + 4],
                                         small[:, INVL:INVL + 4])

                    # inv32 -> [32,128] -> DRAM [4096] -> bcast [128, C]
                    tp = psA.tile([32, 128], f32, tag="gps", name="tpp")
                    nc.tensor.transpose(tp[:], small[:, NSQ32:NSQ32 + 32],
                                        ident[:])
                    tps = gscr.tile([32, 128], f32, tag="tps")
                    nc.scalar.copy(tps[:], tp[:])
                    nrmd = dram_pool.tile([C], f32)
                    nc.sync.dma_start(
                        nrmd[:].rearrange("(j p) -> j p", p=128), tps[:])
                    nr = nrmd[:]
                    nc.gpsimd.dma_start(invb[:], bass.AP(
                        tensor=nr.tensor, offset=nr.offset,
                        ap=[[0, 128]] + [list(p) for p in nr.ap]))

                    # ---- Phase B: sim chunk -> conf (PE -> ACT direct) ----
                    if stage != "norm":
                        for m in range(NG):
                            tmph = [None, None]
                            for n in range(8):
                                g_ps = psA.tile([128, 512], f32, tag="gps")
                                for kd in range(8):
                                    nc.tensor.matmul(
                                        g_ps[:],
                                        wtloc_sb[:, kd * CHUNK + 128 * m:
                                                 kd * CHUNK + 128 * m + 128],
                                        wt_sb[:, kd * C + 512 * n:
                                              kd * C + 512 * n + 512],
                                        start=(kd == 0), stop=(kd == 7))
                                if exp_halves:
                                    h = n // 4
                                    if tmph[h] is None:
                                        tmph[h] = gscr.tile(
                                            [128, 2048], bf16, tag="gtmp",
                                            name=f"tmph{m}_{h}")
                                    nc.vector.tensor_tensor(
                                        tmph[h][:, 512 * (n % 4):
                                                512 * (n % 4) + 512],
                                        g_ps[:],
                                        invb[:, 512 * n:512 * n + 512],
                                        op=AL.mult)
                                    if n % 4 == 3:
                                        nc.scalar.activation(
                                            conf[:, m * C + 2048 * h:
                                                 m * C + 2048 * h + 2048],
                                            tmph[h][:], AF.Exp,
                                            scale=small[:, INVL + m:
                                                        INVL + m + 1],
                                            accum_out=small[
                                                :, ESG + m * 8 + h:
                                                ESG + m * 8 + h + 1])
                                    continue
                                tmp = gscr.tile([128, 512], f32, tag="gtmp")
                                nc.vector.tensor_tensor(
                                    tmp[:], g_ps[:],
                                    invb[:, 512 * n:512 * n + 512],
                                    op=AL.mult)
                                nc.scalar.activation(
                                    conf[:, m * C + 512 * n:
                                         m * C + 512 * n + 512],
                                    tmp[:], AF.Exp,
                                    scale=small[:, INVL + m:INVL + m + 1],
                                    accum_out=small[:, ESG + m * 8 + n:
                                                    ESG + m * 8 + n + 1])
                            # den = rowsum - e ; rden = 1/den
                            nred = 2 if exp_halves else 8
                            nc.vector.reduce_sum(
                                small[:, RDEN + m:RDEN + m + 1],
                                small[:, ESG + m * 8:ESG + m * 8 + nred],
                                axis=mybir.AxisListType.X)
                            nc.vector.tensor_scalar_add(
                                small[:, RDEN + m:RDEN + m + 1],
                                small[:, RDEN + m:RDEN + m + 1], -E_CONST)
                            nc.vector.reciprocal(
                                small[:, RDEN + m:RDEN + m + 1],
                                small[:, RDEN + m:RDEN + m + 1])
                            # conf = e * rden (diag kept; corrected later)
                            nc.vector.tensor_scalar_mul(
                                conf[:, m * C:(m + 1) * C],
                                conf[:, m * C:(m + 1) * C],
                                small[:, RDEN + m:RDEN + m + 1])

                # ============== Phase C: Q per group + lse ==============
                if stage not in ("norm", "g"):
                    with tc.tile_pool(name="psQ",
                                      bufs=(8 if q_split else 1),
                                      space="PSUM") as psQ:
                        for m in range(NG):
                            if q_split:
                                qs = [psQ.tile([128, 512], f32,
                                               name=f"q{m}_{n}", tag="qq")
                                      for n in range(8)]
                            else:
                                q_ps = psQ.tile([128, C], f32)
                            for ki in range(nkt):
                                kt = m * nkt + ki
                                pt = pred_pool.tile([128, C], bf16)
                                nc.sync.dma_start(
                                    pt[:], predb[kt * 128:(kt + 1) * 128, :])
                                oh = oh_pool.tile([128, 128], bf16)
                                oh_eng = (nc.gpsimd if oh_gpsimd
                                          else nc.vector)
                                oh_eng.tensor_scalar(
                                    oh[:], colidx[:], ltg_sb[:, kt:kt + 1],
                                    None, op0=AL.is_equal)
                                for n in range(8):
                                    qdst = (qs[n][:] if q_split else
                                            q_ps[:, 512 * n:512 * n + 512])
                                    nc.tensor.matmul(
                                        qdst, oh[:],
                                        pt[:, 512 * n:512 * n + 512],
                                        start=(ki == 0), stop=(ki == nkt - 1))
                                es = escr_pool.tile([128, C], bf16,
                                                    tag="escr")
                                nc.scalar.activation(
                                    es[:], pt[:], AF.Exp,
                                    accum_out=esums[:, kt:kt + 1])
                            # fold -0.9*pred_t and the conf-diagonal
                            # correction into conf_m: conf_m += s'*dmask_sh
                            # with s' = 9 - e*rden, then the single dot
                            # (coef -0.1) covers all Q terms.
                            nc.vector.tensor_scalar(
                                small[:, SPR + m:SPR + m + 1],
                                small[:, RDEN + m:RDEN + m + 1],
                                -E_CONST, 9.0, op0=AL.mult, op1=AL.add)
                            sdm = escr_pool.tile([128, C], bf16, tag="escr")
                            nc.vector.tensor_scalar(
                                sdm[:, 0:C - 128 * m],
                                dmask_sb[:, 0:C - 128 * m],
                                small[:, SPR + m:SPR + m + 1],
                                None, op0=AL.mult)
                            nc.vector.tensor_tensor(
                                conf[:, m * C + 128 * m:(m + 1) * C],
                                conf[:, m * C + 128 * m:(m + 1) * C],
                                sdm[:, 0:C - 128 * m], op=AL.add)
                            if q_split:
                                for n in range(8):
                                    scr2 = escr_pool.tile(
                                        [128, 512], bf16, tag="dchunk",
                                        name=f"dch{m}_{n}")
                                    nc.vector.tensor_tensor(
                                        scr2[:], qs[n][:],
                                        conf[:, m * C + 512 * n:
                                             m * C + 512 * n + 512],
                                        op=AL.mult)
                                    nc.vector.reduce_sum(
                                        small[:, ESG + m * 8 + n:
                                              ESG + m * 8 + n + 1],
                                        scr2[:], axis=mybir.AxisListType.X)
                                nc.vector.reduce_sum(
                                    small[:, FIN + TK + m:FIN + TK + m + 1],
                                    small[:, ESG + m * 8:ESG + m * 8 + 8],
                                    axis=mybir.AxisListType.X)
                            else:
                                scr2 = escr_pool.tile([128, C], bf16,
                                                      tag="escr")
                                nc.vector.tensor_tensor(
                                    scr2[:], q_ps[:],
                                    conf[:, m * C:(m + 1) * C],
                                    op=AL.mult)
                                nc.vector.reduce_sum(
                                    small[:, FIN + TK + m:FIN + TK + m + 1],
                                    scr2[:], axis=mybir.AxisListType.X)

            # ================= Phase D: final reduction =================
            with tc.tile_pool(name="psF", bufs=1, space="PSUM") as psF:
                outsb = escr_pool.tile([1, 1], f32, tag="outsb")
                if stage == "full":
                    nc.scalar.activation(small[:, LNV:LNV + TK], esums[:],
                                         AF.Ln)
                    nc.vector.tensor_tensor(small[:, FIN:FIN + TK],
                                            small[:, LNV:LNV + TK],
                                            vmask_sb[:], op=AL.mult)
                    nc.vector.memset(small[:, COEF:COEF + TK], 1.0)
                    nc.vector.memset(small[:, COEF + TK:COEF + TK + 4],
                                     -SMOOTHING)
                    nc.vector.tensor_tensor(
                        small[:, PROD:PROD + F],
                        small[:, FIN:FIN + F],
                        small[:, COEF:COEF + F], op=AL.mult)
                    nc.vector.reduce_sum(small[:, OUTC:OUTC + 1],
                                         small[:, PROD:PROD + F],
                                         axis=mybir.AxisListType.X)
                    fps = psF.tile([1, 1], f32)
                    nc.tensor.matmul(fps[:], small[:, OUTC:OUTC + 1],
                                     small[:, ONES:ONES + 1])
                    nc.scalar.copy(outsb[:], fps[:])
                else:
                    nc.vector.memset(outsb[:], 0.0)
                nc.sync.dma_start(out, outsb[:])

    nc.compile()
    if split_drains:
        _split_multiwait_drains(nc)
    return nc


def _prep(pred, weight, target):
    """Host-side sharding/staging. Returns (in_maps, nkt)."""
    pred = np.asarray(pred)
    weight = np.asarray(weight, dtype=np.float32)
    target = np.asarray(target).astype(np.int64)

    w_bf = weight.astype(ml_dtypes.bfloat16)
    wt_bf = np.ascontiguousarray(w_bf.T)

    core_of = (target // CHUNK).astype(np.int64)
    rows_per_core = [np.nonzero(core_of == k)[0] for k in range(NCORES)]

    # group sizes -> uniform tiles per group
    maxg = 1
    groups = []
    for k in range(NCORES):
        lt = target[rows_per_core[k]] - CHUNK * k
        gs = [rows_per_core[k][lt // 128 == m] for m in range(NG)]
        groups.append(gs)
        for g in gs:
            maxg = max(maxg, len(g))
    nkt = (maxg + 127) // 128
    TK = NG * nkt
    S = TK * 128

    pred_bf = pred.astype(ml_dtypes.bfloat16)

    in_maps = []
    for k in range(NCORES):
        predb = np.zeros((S, C), dtype=ml_dtypes.bfloat16)
        ltg = np.full((128, TK), 9999.0, dtype=np.float32)
        vm = np.zeros((128, TK), dtype=np.float32)
        for m in range(NG):
            idx = groups[k][m]
            off = m * nkt * 128
            n = len(idx)
            predb[off:off + n] = pred_bf[idx]
            r = off + np.arange(n)
            ltg[r & 127, r >> 7] = (target[idx] - CHUNK * k - 128 * m)
            vm[r & 127, r >> 7] = 1.0
        dmask = np.zeros((128, C), dtype=ml_dtypes.bfloat16)
        dmask[np.arange(128), CHUNK * k + np.arange(128)] = 1.0
        in_maps.append({
            "predb": predb,
            "wt_all": wt_bf,
            "wt_loc": np.ascontiguousarray(wt_bf[:, CHUNK * k:CHUNK * (k + 1)]),
            "ltg": ltg,
            "vmask": vm,
            "dmask": dmask,
        })
    return in_maps, nkt


def _install_trace_shims():
    """Make trace=True work in containers whose antenv lacks axon_hooks."""
    import sys
    import types
    try:
        import antenv.axon_hooks  # noqa: F401
    except ImportError:
        import antenv
        from trn_agent_boot.trn_boot import _ntff_profile_via_ctypes
        mod = types.ModuleType("antenv.axon_hooks")
        hook = _ntff_profile_via_ctypes("/opt/axon/libaxon_pjrt.so")
        mod.get_axon_ntff_profile_hook = lambda: hook
        mod.set_axon_ntff_profile_hook = lambda h: None
        sys.modules["antenv.axon_hooks"] = mod
        antenv.axon_hooks = mod
    import concourse.bass_utils as bu
    bu.upload_artifacts = lambda tmpdir: "local://" + tmpdir


def kernel(pred, weight, target):
    from concourse.bass_utils import run_bass_kernel_spmd
    global LAST_RESULTS

    in_maps, nkt = _prep(pred, weight, target)
    if nkt not in _cache:
        _cache[nkt] = _build(nkt)
    nc = _cache[nkt]

    trace = bool(int(os.environ.get("AKL_TRACE", "0")))
    if trace:
        _install_trace_shims()
    res = run_bass_kernel_spmd(nc, in_maps, core_ids=list(range(NCORES)),
                               trace=trace)
    LAST_RESULTS = res
    total = np.float64(0.0)
    for k in range(NCORES):
        total += np.float64(res.results[k]["out"][0, 0])
    return np.float32(total / B)
